# revision 25
# baseline (speedup 1.0000x reference)
"""ExpertsChooseMaskedMLP on 8 TRN2 NeuronCores.

Sharding: 4-way expert parallel x 2-way data parallel. Core k owns batch
b = k//4 and experts {2*(k%4), 2*(k%4)+1}. Each core runs the full
dispatch -> contract -> GELU -> expand -> combine pipeline for its two
experts, producing a partial [T, D] output for its batch; chunked
ReduceScatters over each 4-core group (overlapped with the combine
matmuls) sum the expert partials and leave each core with four
[T/16, D] shards of the final output, which the host reassembles.

Per-core dataflow (bf16 operands, f32 PSUM accumulation):
  stage 1: xeT[e] [D,C]  = x[b](lhsT) @ dispatch_e        (K=T)
  stage 2: hT[e]  [O1,C] = gelu(W1T_e(lhsT) @ xeT + b1)   (K=D)
  stage 3: y[e]   [C,D]  = hT(lhsT) @ W2T_e + b2          (K=O1)
  stage 4: part   [T,D]  = sum_e combT_e(lhsT) @ y_e      (K=C)
"""
import numpy as np
import ml_dtypes
from contextlib import ExitStack

import concourse.bass as bass
import concourse.tile as tile
from concourse import bacc, mybir
from concourse import bass_utils

B, T, D, E, C = 2, 2048, 2048, 8, 512
O1 = D // E          # 256
NCORES = 8
GROUPS = [[0, 1, 2, 3], [4, 5, 6, 7]]  # one group per batch
EPC = 2              # experts per core

P = 128
KT = T // P          # 16  (T chunks)
KD = D // P          # 16  (D chunks)
KC = C // P          # 4   (C chunks)
KO = O1 // P         # 2   (O1 chunks)
ND = D // 512        # 4   (D n-tiles)
CHUNKS = [3, 4, 4, 3, 2]  # T-chunks per RS chunk
NRS = len(CHUNKS)
CH0 = [sum(CHUNKS[:g]) for g in range(NRS)]  # first mt of each chunk
OUT0 = [c * P // 4 for c in CH0]             # out-row offset per chunk

F32 = mybir.dt.float32
BF16 = mybir.dt.bfloat16

_STATE = {}


def _build():
    nc = bacc.Bacc("TRN2", target_bir_lowering=False, debug=False,
                   num_devices=NCORES)
    xp = nc.dram_tensor("xp", [KD, P, KT, P], BF16, kind="ExternalInput")
    dp = nc.dram_tensor("dp", [EPC, P, KT, C], BF16, kind="ExternalInput")
    cb = nc.dram_tensor("cb", [EPC, P, KT, C], BF16, kind="ExternalInput")
    w1 = nc.dram_tensor("w1", [EPC, P, KD, O1], BF16, kind="ExternalInput")
    w2 = nc.dram_tensor("w2", [EPC, P, KO, D], BF16, kind="ExternalInput")
    b1p = nc.dram_tensor("b1p", [P, EPC * KO], F32, kind="ExternalInput")
    b2p = nc.dram_tensor("b2p", [P, D], F32, kind="ExternalInput")
    out = nc.dram_tensor("out", [T // 4, D], BF16, kind="ExternalOutput")

    with tile.TileContext(nc) as tc:
        with ExitStack() as ctx:
            dpp = ctx.enter_context(tc.tile_pool(name="dpp", bufs=1))
            cbp = ctx.enter_context(tc.tile_pool(name="cbp", bufs=1))
            bpool = ctx.enter_context(tc.tile_pool(name="bpool", bufs=2))
            xpool = ctx.enter_context(tc.tile_pool(name="xpool", bufs=4))
            wp = ctx.enter_context(tc.tile_pool(name="wp", bufs=1))
            hp = ctx.enter_context(tc.tile_pool(name="hp", bufs=2))
            cpool = ctx.enter_context(tc.tile_pool(name="cpool", bufs=1))
            ps = ctx.enter_context(tc.tile_pool(name="ps", bufs=8, space="PSUM"))
            op = ctx.enter_context(tc.tile_pool(name="op", bufs=8))
            dram = ctx.enter_context(tc.tile_pool(name="dram", bufs=1, space="DRAM"))

            # dispatch resident; split each expert's load across two DMA
            # queues so the kernel-head critical path isn't one-queue-bound
            dr = [dpp.tile([P, KT, C], BF16, tag=f"dr{j}", name=f"dr{j}")
                  for j in range(EPC)]
            Q = KT // 4
            nc.sync.dma_start(dr[0][:, :Q, :], dp[0, :, :Q, :])
            nc.scalar.dma_start(dr[0][:, Q:2 * Q, :], dp[0, :, Q:2 * Q, :])
            nc.gpsimd.dma_start(dr[0][:, 2 * Q:3 * Q, :], dp[0, :, 2 * Q:3 * Q, :])
            nc.scalar.dma_start(dr[0][:, 3 * Q:, :], dp[0, :, 3 * Q:, :])
            nc.gpsimd.dma_start(dr[1][:, :KT // 2, :], dp[1, :, :KT // 2, :])
            nc.scalar.dma_start(dr[1][:, KT // 2:, :], dp[1, :, KT // 2:, :])

            # constants + resident weights/combine: deferred via clock waits
            # so the kernel head's bandwidth goes to dispatch + first x strips
            b1t = cpool.tile([P, EPC * KO], F32, tag="b1")
            nc.gpsimd.dma_start(b1t[:], b1p[:, :])
            b2t = cpool.tile([P, D], F32, tag="b2")
            w1r = wp.tile([P, EPC, KD, O1], BF16, tag="w1")
            w2r = wp.tile([P, EPC, KO, D], BF16, tag="w2")
            with tc.tile_wait_until(0.040):
                nc.scalar.dma_start(b2t[:], b2p[:, :])
            with tc.tile_wait_until(0.050):
                nc.scalar.dma_start(w1r[:, 0, :, :], w1[0, :, :, :])
            with tc.tile_wait_until(0.060):
                nc.scalar.dma_start(w1r[:, 1, :, :], w1[1, :, :, :])
                nc.scalar.dma_start(w2r[:, 0, :, :], w2[0, :, :, :])
            with tc.tile_wait_until(0.070):
                nc.scalar.dma_start(w2r[:, 1, :, :], w2[1, :, :, :])

            cbr = [cbp.tile([P, KT, C], BF16, tag=f"cb{j}", name=f"cb{j}")
                   for j in range(EPC)]
            with tc.tile_wait_until(0.080):
                nc.gpsimd.dma_start(cbr[0][:], cb[0, :, :, :])
            with tc.tile_wait_until(0.095):
                nc.gpsimd.dma_start(cbr[1][:], cb[1, :, :, :])

            # ---- stage 1: xeT[j] [D(part chunks m), C] ----
            xeT = [bpool.tile([P, KD, C], BF16, tag="bigB", name=f"xeT{j}")
                   for j in range(EPC)]
            xr_dmas = []
            for m in range(KD):
                xr = xpool.tile([P, KT, P], BF16, tag="xr")
                xr_dmas.append(nc.sync.dma_start(xr[:], xp[m, :, :, :]))
                for j in range(EPC):
                    pt = ps.tile([P, C], F32, tag="ps")
                    for kt in range(KT):
                        nc.tensor.matmul(pt[:], xr[:, kt, :], dr[j][:, kt, :],
                                         start=(kt == 0), stop=(kt == KT - 1))
                    nc.vector.tensor_copy(xeT[j][:, m, :], pt[:])

            # ---- stage 2: hT[j] [O1(part chunks mo), C] = gelu(W1T @ xeT + b1)
            hT = [hp.tile([P, KO, C], BF16, tag="hT", name=f"hT{j}")
                  for j in range(EPC)]
            for j in range(EPC):
                pts = [ps.tile([P, C], F32, tag="ps", name=f"pt{mo}")
                       for mo in range(KO)]
                for kd in range(KD):
                    for mo in range(KO):
                        nc.tensor.matmul(
                            pts[mo][:],
                            w1r[:, j, kd, mo * P:(mo + 1) * P],
                            xeT[j][:, kd, :],
                            start=(kd == 0), stop=(kd == KD - 1))
                for mo in range(KO):
                    nc.scalar.activation(hT[j][:, mo, :], pts[mo][:],
                                         mybir.ActivationFunctionType.Gelu,
                                         bias=b1t[:, j * KO + mo:j * KO + mo + 1])

            # ---- stage 3: y[j] [C(part chunks mc), D] = hT @ W2T + b2 ----
            y = [bpool.tile([P, KC, D], BF16, tag="bigB", name=f"y{j}")
                 for j in range(EPC)]
            for j in range(EPC):
                for mc in range(KC):
                    for nd in range(ND):
                        pt = ps.tile([P, 512], F32, tag="ps")
                        for ko in range(KO):
                            nc.tensor.matmul(
                                pt[:],
                                hT[j][:, ko, mc * P:(mc + 1) * P],
                                w2r[:, j, ko, nd * 512:(nd + 1) * 512],
                                start=(ko == 0), stop=(ko == KO - 1))
                        nc.vector.tensor_add(y[j][:, mc, nd * 512:(nd + 1) * 512],
                                             pt[:], b2t[:, nd * 512:(nd + 1) * 512])

            # ---- stage 4 + chunked reduce-scatter ----
            rs_in = [dram.tile([CHUNKS[g] * P, D], BF16, name=f"rs_in{g}")
                     for g in range(NRS)]
            rs_out = [dram.tile([CHUNKS[g] * P // 4, D], BF16, name=f"rs_out{g}")
                      for g in range(NRS)]
            for g in range(NRS):
                for mi in range(CHUNKS[g]):
                    mt = CH0[g] + mi
                    ot = op.tile([P, ND * 512], BF16, tag="ot")
                    for nd in range(ND):
                        pt = ps.tile([P, 512], F32, tag="ps")
                        for j in range(EPC):
                            for kc in range(KC):
                                nc.tensor.matmul(
                                    pt[:],
                                    cbr[j][:, mt, kc * P:(kc + 1) * P],
                                    y[j][:, kc, nd * 512:(nd + 1) * 512],
                                    start=(j == 0 and kc == 0),
                                    stop=(j == EPC - 1 and kc == KC - 1))
                        if nd % 2 == 0:
                            nc.scalar.activation(
                                ot[:, nd * 512:(nd + 1) * 512], pt[:],
                                mybir.ActivationFunctionType.Copy)
                        else:
                            nc.vector.tensor_copy(
                                ot[:, nd * 512:(nd + 1) * 512], pt[:])
                    eng = nc.sync if mt % 2 == 0 else nc.scalar
                    eng.dma_start(rs_in[g][mi * P:(mi + 1) * P, :], ot[:])
                nc.gpsimd.collective_compute(
                    "ReduceScatter",
                    mybir.AluOpType.add,
                    replica_groups=GROUPS,
                    ins=[rs_in[g][:].opt()],
                    outs=[rs_out[g][:].opt()],
                )
                nc.sync.dma_start(
                    out[OUT0[g]:OUT0[g] + CHUNKS[g] * P // 4, :], rs_out[g][:])
    nc.compile()
    return nc


def _get_nc():
    if "nc" not in _STATE:
        _STATE["nc"] = _build()
    return _STATE["nc"]


def _prep_inputs(x, dispatch_mask, combine_array, W1, b1, W2, b2):
    x = np.asarray(x, dtype=np.float32)
    dispatch_mask = np.asarray(dispatch_mask, dtype=np.float32)
    combine_array = np.asarray(combine_array, dtype=np.float32)
    W1 = np.asarray(W1, dtype=np.float32)
    b1 = np.asarray(b1, dtype=np.float32)
    W2 = np.asarray(W2, dtype=np.float32)
    b2 = np.asarray(b2, dtype=np.float32)
    bf = ml_dtypes.bfloat16

    b2p = np.ascontiguousarray(np.broadcast_to(b2, (P, D)))
    in_maps = []
    for k in range(NCORES):
        b = k // 4
        e0 = EPC * (k % 4)
        # x[b] [T,D] -> [m(KD), p, kt, dcol]
        xp = np.ascontiguousarray(
            x[b].reshape(KT, P, KD, P).transpose(2, 1, 0, 3)).astype(bf)
        # dispatch [T,C] per expert -> [j, p, kt, c]
        dp = np.ascontiguousarray(
            dispatch_mask[b, :, e0:e0 + EPC, :].reshape(KT, P, EPC, C)
            .transpose(2, 1, 0, 3)).astype(bf)
        # combine [T,C] per expert -> [j, p(c), mt, kc*128+tcol]
        cbs = []
        for j in range(EPC):
            a = combine_array[b, :, e0 + j, :].reshape(KT, P, KC, P)
            cbs.append(a.transpose(3, 0, 2, 1).reshape(P, KT, C))
        cbp = np.ascontiguousarray(np.stack(cbs)).astype(bf)
        # W1[e] [O1,D] -> W1T [D,O1] -> [j, p, kd, o]
        w1p = np.ascontiguousarray(
            W1[e0:e0 + EPC].transpose(0, 2, 1).reshape(EPC, KD, P, O1)
            .transpose(0, 2, 1, 3)).astype(bf)
        # W2[e] [D,O1] -> W2T [O1,D] -> [j, p, ko, d]
        w2p = np.ascontiguousarray(
            W2[e0:e0 + EPC].transpose(0, 2, 1).reshape(EPC, KO, P, D)
            .transpose(0, 2, 1, 3)).astype(bf)
        # b1 [O1] per expert -> [p, j*KO+mo]
        b1p = np.ascontiguousarray(
            b1[e0:e0 + EPC].reshape(EPC, KO, P).transpose(2, 0, 1)
            .reshape(P, EPC * KO))
        in_maps.append(dict(xp=xp, dp=dp, cb=cbp, w1=w1p, w2=w2p,
                            b1p=b1p, b2p=b2p))
    return in_maps


def _assemble(results):
    out = np.empty((B, T, D), dtype=np.float32)
    for k in range(NCORES):
        b = k // 4
        i = k % 4
        o = results[k]["out"].astype(np.float32)
        for g in range(NRS):
            rows = CHUNKS[g] * P // 4
            r0 = CH0[g] * P + i * rows
            out[b, r0:r0 + rows, :] = o[OUT0[g]:OUT0[g] + rows]
    return out


def _run(in_maps, trace=False):
    nc = _get_nc()
    res = bass_utils.run_bass_kernel_spmd(
        nc, in_maps, core_ids=list(range(NCORES)), trace=trace)
    return res


def kernel(x, dispatch_mask, combine_array, W1, b1, W2, b2):
    in_maps = _prep_inputs(x, dispatch_mask, combine_array, W1, b1, W2, b2)
    last = None
    for wait in (0, 30, 90):
        if wait:
            import time as _time
            _time.sleep(wait)
        try:
            res = _run(in_maps, trace=False)
            return _assemble(res.results)
        except Exception as e:  # transient runtime hiccups (worker restart)
            last = e
    raise last
